# revision 7
# baseline (speedup 1.0000x reference)
"""Cross-attention-with-bias Trainium2 kernel (8-core SPMD).

Problem: out = MHA(query, key, value, key_padding_mask) with torch-style
Linear projections (x @ W.T + b), H=16 heads, D=64, E=1024, B=2, Q=1024,
K=2048, fp32.

Sharding: 8 cores = (batch b in {0,1}) x (head-group g in {0..3}, 4 heads
each).  Each core computes its batch's q/k/v projections restricted to its
256 output features, attention for its 4 heads, and a partial output
projection (contraction over its 256 features).  Host sums the 4 partials
per batch and adds the combined bias.

Device-side layout choices (all picked so no activation transposes are
needed on device):
  - host passes query/key/value transposed: [E, T] (contraction dim on
    partitions for the projection matmuls)
  - projections produce q^T,k^T in [feature, token] layout, v in natural
    [token, feature] layout
  - logits are computed transposed: S^T[k_tok, q_tok] so the padding-mask
    bias is a per-partition scalar folded into the Exp activation
    (exp(scale*S + bias), bias = -50 for masked keys); no softmax max
    subtraction (logits are provably < ~16 in magnitude * scale)
  - v gets a ones-column appended per head, so the PV matmul also yields
    the softmax denominators (row 64 of the [65, Q] accumulator)
  - normalization: denominators -> reciprocal -> broadcast across 64
    partitions with a tiny indicator matmul -> elementwise multiply
  - v bias (bv) never touches the device: attn rows sum to 1, so
    attn@(v+bv) = attn@v + bv, and bv @ Wo.T + bo is added on host.
"""

import sys

sys.path.insert(0, "/opt/trn_rl_repo")

import numpy as np

import concourse.bass as bass  # noqa: F401  (registers types)
import concourse.tile as tile
from concourse import bacc, mybir
from concourse import bass_utils

P = 128
B, Q, KT, E = 2, 1024, 2048, 1024
H, D = 16, 64
G = 4            # head groups (cores per batch)
HG = H // G      # heads per core = 4
F = HG * D       # feature slice per core = 256
SCALE = float(D) ** -0.5
MASK_BIAS = -50.0
N_CORES = 8
FP = mybir.dt.float32


def _build_body(nc, tc, pools, dram, rep):
    """One full iteration of the per-core computation."""
    wq_sb, wk_sb, wv_sb, wo_sb, maskb_sb, bq_sb, bk_sb, ones64 = pools["consts"]
    res, io, pt_pool, ypool, ps_a, ps_pv = (
        pools["res"], pools["io"], pools["pt"], pools["y"], pools["ps_a"], pools["ps_pv"],
    )
    qT, kT, vT, y = dram["qT"], dram["kT"], dram["vT"], dram["y"]
    Exp = mybir.ActivationFunctionType.Exp

    # per-iteration resident tiles
    qT_sb = res.tile([P, 2, Q], FP, tag="qt", name=f"qT_sb_{rep}")
    kT_sb = res.tile([P, 2, KT], FP, tag="kt", name=f"kT_sb_{rep}")
    v_sb = res.tile([P, 16, HG * (D + 1)], FP, tag="v", name=f"v_sb_{rep}")
    un = res.tile([P, 2, Q], FP, tag="un", name=f"un_{rep}")
    nm = res.tile([P, 2, Q], FP, tag="nm", name=f"nm_{rep}")
    dnr = [res.tile([1, Q], FP, tag=f"dnr{h}", name=f"dnr_{rep}_{h}") for h in range(HG)]

    # ---- Phase 1a: q projection (2 blocks of 512 tokens) ----
    for tb in range(2):
        qin = io.tile([P, 8, 512], FP, tag="in", name=f"qin_{rep}_{tb}")
        nc.sync.dma_start(
            qin[:], qT[:, tb * 512:(tb + 1) * 512].rearrange("(c p) t -> p c t", p=P)
        )
        ps_q = ps_a.tile([P, Q], FP, tag="t", name=f"ps_q_{rep}_{tb}")
        for i in range(8):
            for j in range(2):
                nc.tensor.matmul(
                    ps_q[:, j * 512:(j + 1) * 512],
                    lhsT=wq_sb[:, i, j * P:(j + 1) * P],
                    rhs=qin[:, i, :],
                    start=(i == 0),
                    stop=(i == 7),
                )
        for j in range(2):
            nc.vector.tensor_scalar_add(
                qT_sb[:, j, tb * 512:(tb + 1) * 512],
                ps_q[:, j * 512:(j + 1) * 512],
                bq_sb[:, j:j + 1],
            )

    # ---- Phase 1b: k/v projections (4 blocks of 512 kv tokens) ----
    for tb in range(4):
        kin = io.tile([P, 8, 512], FP, tag="in", name=f"kin_{rep}_{tb}")
        nc.sync.dma_start(
            kin[:], kT[:, tb * 512:(tb + 1) * 512].rearrange("(c p) t -> p c t", p=P)
        )
        ps_k = ps_a.tile([P, Q], FP, tag="t", name=f"ps_k_{rep}_{tb}")
        for i in range(8):
            for j in range(2):
                nc.tensor.matmul(
                    ps_k[:, j * 512:(j + 1) * 512],
                    lhsT=wk_sb[:, i, j * P:(j + 1) * P],
                    rhs=kin[:, i, :],
                    start=(i == 0),
                    stop=(i == 7),
                )
        for j in range(2):
            nc.vector.tensor_scalar_add(
                kT_sb[:, j, tb * 512:(tb + 1) * 512],
                ps_k[:, j * 512:(j + 1) * 512],
                bk_sb[:, j:j + 1],
            )

        vin = io.tile([P, 8, 512], FP, tag="in", name=f"vin_{rep}_{tb}")
        nc.sync.dma_start(
            vin[:], vT[:, tb * 512:(tb + 1) * 512].rearrange("(c p) t -> p c t", p=P)
        )
        ps_v = ps_pv.tile([P, Q], FP, tag="t", name=f"ps_v_{rep}_{tb}")
        for tt4 in range(4):
            for i in range(8):
                nc.tensor.matmul(
                    ps_v[:, tt4 * F:(tt4 + 1) * F],
                    lhsT=vin[:, i, tt4 * P:(tt4 + 1) * P],
                    rhs=wv_sb[:, i, :],
                    start=(i == 0),
                    stop=(i == 7),
                )
        for tt4 in range(4):
            kt = 4 * tb + tt4
            vslot = v_sb[:, kt].rearrange("p (h x) -> p h x", h=HG)
            nc.vector.memset(vslot[:, :, D:D + 1], 1.0)
            nc.vector.tensor_copy(
                out=vslot[:, :, 0:D],
                in_=ps_v[:, tt4 * F:(tt4 + 1) * F].rearrange("p (h d) -> p h d", h=HG),
            )

    # ---- Phase 2: attention per head ----
    for h in range(HG):
        jh = h // 2
        ph = 64 * (h % 2)
        ps_o = ps_pv.tile([D + 1, Q], FP, tag="t", name=f"ps_o_{rep}_{h}")
        for kt in range(16):
            ps_s = ps_a.tile([P, Q], FP, tag="t", name=f"ps_s_{rep}_{h}_{kt}")
            for qh in range(2):
                nc.tensor.matmul(
                    ps_s[:, qh * 512:(qh + 1) * 512],
                    lhsT=kT_sb[ph:ph + D, jh, kt * P:(kt + 1) * P],
                    rhs=qT_sb[ph:ph + D, jh, qh * 512:(qh + 1) * 512],
                    start=True,
                    stop=True,
                )
            pt = pt_pool.tile([P, Q], FP, tag="pt", name=f"pt_{rep}_{h}_{kt}")
            nc.scalar.activation(
                pt[:], ps_s[:], Exp, bias=maskb_sb[:, kt:kt + 1], scale=SCALE
            )
            for qh in range(2):
                nc.tensor.matmul(
                    ps_o[:, qh * 512:(qh + 1) * 512],
                    lhsT=v_sb[:, kt, h * (D + 1):(h + 1) * (D + 1)],
                    rhs=pt[:, qh * 512:(qh + 1) * 512],
                    start=(kt == 0),
                    stop=(kt == 15),
                )
        nc.vector.tensor_copy(out=un[ph:ph + D, jh, :], in_=ps_o[0:D, :])
        nc.vector.reciprocal(dnr[h][:], ps_o[D:D + 1, :])

    # ---- Normalization: broadcast 1/denom across 64 partitions via K=1 matmul ----
    for pr in range(2):
        ps_b = ps_a.tile([P, Q], FP, tag="t", name=f"ps_b_{rep}_{pr}")
        for par in range(2):
            h = 2 * pr + par
            for qh in range(2):
                nc.tensor.matmul(
                    ps_b[par * D:(par + 1) * D, qh * 512:(qh + 1) * 512],
                    lhsT=ones64[:, :],
                    rhs=dnr[h][:, qh * 512:(qh + 1) * 512],
                    start=True,
                    stop=True,
                )
        nc.vector.tensor_mul(out=nm[:, pr, :], in0=un[:, pr, :], in1=ps_b[:])

    # ---- Phase 3: output projection (partial; host adds bias) ----
    for tt in range(8):
        ps_y = ps_a.tile([P, Q], FP, tag="t", name=f"ps_y_{rep}_{tt}")
        for pr in range(2):
            for c in range(2):
                nc.tensor.matmul(
                    ps_y[:, c * 512:(c + 1) * 512],
                    lhsT=nm[:, pr, tt * P:(tt + 1) * P],
                    rhs=wo_sb[:, pr, c * 512:(c + 1) * 512],
                    start=(pr == 0),
                    stop=(pr == 1),
                )
        yt = ypool.tile([P, Q], FP, tag="y", name=f"yt_{rep}_{tt}")
        nc.vector.tensor_copy(out=yt[:], in_=ps_y[:])
        nc.sync.dma_start(y[tt * P:(tt + 1) * P, :], yt[:])


def build_module(repeats=1):
    """Build and compile the per-core Bass module. Returns the compiled nc."""
    nc = bacc.Bacc(
        "TRN2",
        target_bir_lowering=False,
        debug=False,
        enable_asserts=False,
        num_devices=N_CORES,
    )
    dram = {
        "qT": nc.dram_tensor("qT", [E, Q], FP, kind="ExternalInput").ap(),
        "kT": nc.dram_tensor("kT", [E, KT], FP, kind="ExternalInput").ap(),
        "vT": nc.dram_tensor("vT", [E, KT], FP, kind="ExternalInput").ap(),
        "wq": nc.dram_tensor("wq", [E, F], FP, kind="ExternalInput").ap(),
        "wk": nc.dram_tensor("wk", [E, F], FP, kind="ExternalInput").ap(),
        "wv": nc.dram_tensor("wv", [E, F], FP, kind="ExternalInput").ap(),
        "wo": nc.dram_tensor("wo", [F, E], FP, kind="ExternalInput").ap(),
        "bq2": nc.dram_tensor("bq2", [P, 2], FP, kind="ExternalInput").ap(),
        "bk2": nc.dram_tensor("bk2", [P, 2], FP, kind="ExternalInput").ap(),
        "maskb": nc.dram_tensor("maskb", [P, 16], FP, kind="ExternalInput").ap(),
        "y": nc.dram_tensor("y", [Q, E], FP, kind="ExternalOutput").ap(),
    }

    with tile.TileContext(nc) as tc:
        with (
            tc.tile_pool(name="w", bufs=1) as wpool,
            tc.tile_pool(name="res", bufs=1) as res,
            tc.tile_pool(name="io", bufs=3) as io,
            tc.tile_pool(name="pt", bufs=3) as pt_pool,
            tc.tile_pool(name="yp", bufs=2) as ypool,
            tc.tile_pool(name="psa", bufs=2, space="PSUM") as ps_a,
            tc.tile_pool(name="pspv", bufs=2, space="PSUM") as ps_pv,
        ):
            # constants / weights (loaded once)
            wq_sb = wpool.tile([P, 8, F], FP, name="wq_sb")
            wk_sb = wpool.tile([P, 8, F], FP, name="wk_sb")
            wv_sb = wpool.tile([P, 8, F], FP, name="wv_sb")
            wo_sb = wpool.tile([P, 2, E], FP, name="wo_sb")
            maskb_sb = wpool.tile([P, 16], FP, name="maskb_sb")
            bq_sb = wpool.tile([P, 2], FP, name="bq_sb")
            bk_sb = wpool.tile([P, 2], FP, name="bk_sb")
            ones64 = wpool.tile([1, D], FP, name="ones64")

            nc.sync.dma_start(wq_sb[:], dram["wq"].rearrange("(c p) f -> p c f", p=P))
            nc.sync.dma_start(wk_sb[:], dram["wk"].rearrange("(c p) f -> p c f", p=P))
            nc.sync.dma_start(wv_sb[:], dram["wv"].rearrange("(c p) f -> p c f", p=P))
            nc.sync.dma_start(wo_sb[:], dram["wo"].rearrange("(c p) f -> p c f", p=P))
            nc.sync.dma_start(maskb_sb[:], dram["maskb"][:])
            nc.sync.dma_start(bq_sb[:], dram["bq2"][:])
            nc.sync.dma_start(bk_sb[:], dram["bk2"][:])

            nc.vector.memset(ones64[:], 1.0)

            pools = {
                "consts": (wq_sb, wk_sb, wv_sb, wo_sb, maskb_sb, bq_sb, bk_sb, ones64),
                "res": res, "io": io, "pt": pt_pool, "y": ypool,
                "ps_a": ps_a, "ps_pv": ps_pv,
            }
            for rep in range(repeats):
                _build_body(nc, tc, pools, dram, rep)

    nc.compile()
    return nc


def _make_in_maps(query, key, value, key_padding_mask, Wq, bq, Wk, bk, Wv, bv, Wo, bo):
    """Host-side sharding: returns (in_maps list of 8 dicts, bias_total)."""
    f32 = np.float32
    query = np.asarray(query, f32)
    key = np.asarray(key, f32)
    value = np.asarray(value, f32)
    mask = np.asarray(key_padding_mask)
    Wq, Wk, Wv, Wo = (np.asarray(w, f32) for w in (Wq, Wk, Wv, Wo))
    bq, bk, bv, bo = (np.asarray(b, f32) for b in (bq, bk, bv, bo))

    c = np.ascontiguousarray
    qT = [c(query[b].T) for b in range(B)]
    kTb = [c(key[b].T) for b in range(B)]
    vTb = [c(value[b].T) for b in range(B)]
    maskb = [
        c(np.where(mask[b], f32(MASK_BIAS), f32(0.0)).astype(f32).reshape(16, P).T)
        for b in range(B)
    ]
    in_maps = []
    for core in range(N_CORES):
        b, g = divmod(core, G)
        sl = slice(g * F, (g + 1) * F)
        in_maps.append({
            "qT": qT[b], "kT": kTb[b], "vT": vTb[b],
            "wq": c(Wq[sl, :].T), "wk": c(Wk[sl, :].T), "wv": c(Wv[sl, :].T),
            "wo": c(Wo[:, sl].T),
            "bq2": c(bq[sl].reshape(2, P).T), "bk2": c(bk[sl].reshape(2, P).T),
            "maskb": maskb[b],
        })
    bias_total = (bo + bv @ Wo.T).astype(f32)
    return in_maps, bias_total


_CACHED_NC = None


def kernel(**inputs) -> np.ndarray:
    global _CACHED_NC
    if _CACHED_NC is None:
        _CACHED_NC = build_module(repeats=1)
    nc = _CACHED_NC

    in_maps, bias_total = _make_in_maps(**inputs)
    res = bass_utils.run_bass_kernel_spmd(
        nc, in_maps, core_ids=list(range(N_CORES)), trace=False
    )
    out = np.empty((B, Q, E), np.float32)
    for b in range(B):
        acc = res.results[b * G]["y"].astype(np.float32)
        for g in range(1, G):
            acc = acc + res.results[b * G + g]["y"]
        out[b] = acc + bias_total[None, :]
    return out


# revision 9
# speedup vs baseline: 18.8447x; 18.8447x over previous
"""Cross-attention-with-bias Trainium2 kernel (8-core SPMD).

Problem: out = MHA(query, key, value, key_padding_mask) with torch-style
Linear projections (x @ W.T + b), H=16 heads, D=64, E=1024, B=2, Q=1024,
K=2048, fp32.

Sharding: 8 cores = (batch b in {0,1}) x (head-group g in {0..3}, 4 heads
each).  Each core computes its batch's q/k/v projections restricted to its
256 output features, attention for its 4 heads, and a partial output
projection (contraction over its 256 features).  Host sums the 4 partials
per batch and adds the combined bias.

Device-side layout choices (all picked so no activation transposes are
needed on device):
  - host passes query/key/value transposed: [E, T] (contraction dim on
    partitions for the projection matmuls)
  - projections produce q^T,k^T in [feature, token] layout, v in natural
    [token, feature] layout
  - logits are computed transposed: S^T[k_tok, q_tok] so the padding-mask
    bias is a per-partition scalar folded into the Exp activation
    (exp(scale*S + bias), bias = -50 for masked keys); no softmax max
    subtraction (logits are provably < ~16 in magnitude * scale)
  - v gets a ones-column appended per head, so the PV matmul also yields
    the softmax denominators (row 64 of the [65, Q] accumulator)
  - normalization: denominators -> reciprocal -> broadcast across 64
    partitions with a tiny indicator matmul -> elementwise multiply
  - v bias (bv) never touches the device: attn rows sum to 1, so
    attn@(v+bv) = attn@v + bv, and bv @ Wo.T + bo is added on host.
"""

import sys

sys.path.insert(0, "/opt/trn_rl_repo")

import numpy as np

import concourse.bass as bass  # noqa: F401  (registers types)
import concourse.tile as tile
from concourse import bacc, mybir
from concourse import bass_utils

P = 128
B, Q, KT, E = 2, 1024, 2048, 1024
H, D = 16, 64
G = 4            # head groups (cores per batch)
HG = H // G      # heads per core = 4
F = HG * D       # feature slice per core = 256
SCALE = float(D) ** -0.5
MASK_BIAS = -50.0
N_CORES = 8
FP = mybir.dt.float32


def _build_body(nc, tc, pools, dram, rep):
    """One full iteration of the per-core computation."""
    wq_sb, wk_sb, wv_sb, wo_sb, maskb_sb, bq_sb, bk_sb, ones64 = pools["consts"]
    res, io, pt_pool, ypool, ps_a, ps_pv = (
        pools["res"], pools["io"], pools["pt"], pools["y"], pools["ps_a"], pools["ps_pv"],
    )
    qT, kT, vT, y = dram["qT"], dram["kT"], dram["vT"], dram["y"]
    Exp = mybir.ActivationFunctionType.Exp

    # per-iteration resident tiles
    qT_sb = res.tile([P, 2, Q], FP, tag="qt", name=f"qT_sb_{rep}")
    kT_sb = res.tile([P, 2, KT], FP, tag="kt", name=f"kT_sb_{rep}")
    v_sb = res.tile([P, 16, HG * (D + 1)], FP, tag="v", name=f"v_sb_{rep}")
    un = res.tile([P, 2, Q], FP, tag="un", name=f"un_{rep}")
    nm = res.tile([P, 2, Q], FP, tag="nm", name=f"nm_{rep}")
    dnr = [res.tile([1, Q], FP, tag=f"dnr{h}", name=f"dnr_{rep}_{h}") for h in range(HG)]

    # ---- Phase 1a: q projection (2 blocks of 512 tokens) ----
    for tb in range(2):
        qin = io.tile([P, 8, 512], FP, tag="in", name=f"qin_{rep}_{tb}")
        nc.sync.dma_start(
            qin[:], qT[:, tb * 512:(tb + 1) * 512].rearrange("(c p) t -> p c t", p=P)
        )
        ps_q = ps_a.tile([P, Q], FP, tag="t", name=f"ps_q_{rep}_{tb}")
        for i in range(8):
            for j in range(2):
                nc.tensor.matmul(
                    ps_q[:, j * 512:(j + 1) * 512],
                    lhsT=wq_sb[:, i, j * P:(j + 1) * P],
                    rhs=qin[:, i, :],
                    start=(i == 0),
                    stop=(i == 7),
                )
        for j in range(2):
            nc.vector.tensor_scalar_add(
                qT_sb[:, j, tb * 512:(tb + 1) * 512],
                ps_q[:, j * 512:(j + 1) * 512],
                bq_sb[:, j:j + 1],
            )

    # ---- Phase 1b: k/v projections (4 blocks of 512 kv tokens) ----
    for tb in range(4):
        kin = io.tile([P, 8, 512], FP, tag="in", name=f"kin_{rep}_{tb}")
        nc.sync.dma_start(
            kin[:], kT[:, tb * 512:(tb + 1) * 512].rearrange("(c p) t -> p c t", p=P)
        )
        ps_k = ps_a.tile([P, Q], FP, tag="t", name=f"ps_k_{rep}_{tb}")
        for i in range(8):
            for j in range(2):
                nc.tensor.matmul(
                    ps_k[:, j * 512:(j + 1) * 512],
                    lhsT=wk_sb[:, i, j * P:(j + 1) * P],
                    rhs=kin[:, i, :],
                    start=(i == 0),
                    stop=(i == 7),
                )
        for j in range(2):
            nc.vector.tensor_scalar_add(
                kT_sb[:, j, tb * 512:(tb + 1) * 512],
                ps_k[:, j * 512:(j + 1) * 512],
                bk_sb[:, j:j + 1],
            )

        vin = io.tile([P, 8, 512], FP, tag="in", name=f"vin_{rep}_{tb}")
        nc.sync.dma_start(
            vin[:], vT[:, tb * 512:(tb + 1) * 512].rearrange("(c p) t -> p c t", p=P)
        )
        ps_v = ps_pv.tile([P, Q], FP, tag="t", name=f"ps_v_{rep}_{tb}")
        for tt4 in range(4):
            for i in range(8):
                nc.tensor.matmul(
                    ps_v[:, tt4 * F:(tt4 + 1) * F],
                    lhsT=vin[:, i, tt4 * P:(tt4 + 1) * P],
                    rhs=wv_sb[:, i, :],
                    start=(i == 0),
                    stop=(i == 7),
                )
        for tt4 in range(4):
            kt = 4 * tb + tt4
            vslot = v_sb[:, kt].rearrange("p (h x) -> p h x", h=HG)
            nc.vector.memset(vslot[:, :, D:D + 1], 1.0)
            nc.vector.tensor_copy(
                out=vslot[:, :, 0:D],
                in_=ps_v[:, tt4 * F:(tt4 + 1) * F].rearrange("p (h d) -> p h d", h=HG),
            )

    # ---- Phase 2: attention per head ----
    for h in range(HG):
        jh = h // 2
        ph = 64 * (h % 2)
        ps_o = ps_pv.tile([D + 1, Q], FP, tag="t", name=f"ps_o_{rep}_{h}")
        for kt in range(16):
            ps_s = ps_a.tile([P, Q], FP, tag="t", name=f"ps_s_{rep}_{h}_{kt}")
            for qh in range(2):
                nc.tensor.matmul(
                    ps_s[:, qh * 512:(qh + 1) * 512],
                    lhsT=kT_sb[ph:ph + D, jh, kt * P:(kt + 1) * P],
                    rhs=qT_sb[ph:ph + D, jh, qh * 512:(qh + 1) * 512],
                    start=True,
                    stop=True,
                )
            pt = pt_pool.tile([P, Q], FP, tag="pt", name=f"pt_{rep}_{h}_{kt}")
            nc.scalar.activation(
                pt[:], ps_s[:], Exp, bias=maskb_sb[:, kt:kt + 1], scale=SCALE
            )
            for qh in range(2):
                nc.tensor.matmul(
                    ps_o[:, qh * 512:(qh + 1) * 512],
                    lhsT=v_sb[:, kt, h * (D + 1):(h + 1) * (D + 1)],
                    rhs=pt[:, qh * 512:(qh + 1) * 512],
                    start=(kt == 0),
                    stop=(kt == 15),
                )
        nc.vector.tensor_copy(out=un[ph:ph + D, jh, :], in_=ps_o[0:D, :])
        nc.vector.reciprocal(dnr[h][:], ps_o[D:D + 1, :])

    # ---- Normalization: broadcast 1/denom across 64 partitions via K=1 matmul ----
    for pr in range(2):
        ps_b = ps_a.tile([P, Q], FP, tag="t", name=f"ps_b_{rep}_{pr}")
        for par in range(2):
            h = 2 * pr + par
            for qh in range(2):
                nc.tensor.matmul(
                    ps_b[par * D:(par + 1) * D, qh * 512:(qh + 1) * 512],
                    lhsT=ones64[:, :],
                    rhs=dnr[h][:, qh * 512:(qh + 1) * 512],
                    start=True,
                    stop=True,
                )
        nc.vector.tensor_mul(out=nm[:, pr, :], in0=un[:, pr, :], in1=ps_b[:])

    # ---- Phase 3: output projection (partial; host adds bias) ----
    for tt in range(8):
        ps_y = ps_a.tile([P, Q], FP, tag="t", name=f"ps_y_{rep}_{tt}")
        for pr in range(2):
            for c in range(2):
                nc.tensor.matmul(
                    ps_y[:, c * 512:(c + 1) * 512],
                    lhsT=nm[:, pr, tt * P:(tt + 1) * P],
                    rhs=wo_sb[:, pr, c * 512:(c + 1) * 512],
                    start=(pr == 0),
                    stop=(pr == 1),
                )
        yt = ypool.tile([P, Q], FP, tag="y", name=f"yt_{rep}_{tt}")
        nc.vector.tensor_copy(out=yt[:], in_=ps_y[:])
        nc.sync.dma_start(y[tt * P:(tt + 1) * P, :], yt[:])


def build_module(repeats=1, loop_n=None):
    """Build and compile the per-core Bass module. Returns the compiled nc.

    loop_n: if set, wrap the body in an on-device For loop executing it
    loop_n times (for timing measurements)."""
    nc = bacc.Bacc(
        "TRN2",
        target_bir_lowering=False,
        debug=False,
        enable_asserts=False,
        num_devices=N_CORES,
    )
    dram = {
        "qT": nc.dram_tensor("qT", [E, Q], FP, kind="ExternalInput").ap(),
        "kT": nc.dram_tensor("kT", [E, KT], FP, kind="ExternalInput").ap(),
        "vT": nc.dram_tensor("vT", [E, KT], FP, kind="ExternalInput").ap(),
        "wq": nc.dram_tensor("wq", [E, F], FP, kind="ExternalInput").ap(),
        "wk": nc.dram_tensor("wk", [E, F], FP, kind="ExternalInput").ap(),
        "wv": nc.dram_tensor("wv", [E, F], FP, kind="ExternalInput").ap(),
        "wo": nc.dram_tensor("wo", [F, E], FP, kind="ExternalInput").ap(),
        "bq2": nc.dram_tensor("bq2", [P, 2], FP, kind="ExternalInput").ap(),
        "bk2": nc.dram_tensor("bk2", [P, 2], FP, kind="ExternalInput").ap(),
        "maskb": nc.dram_tensor("maskb", [P, 16], FP, kind="ExternalInput").ap(),
        "y": nc.dram_tensor("y", [Q, E], FP, kind="ExternalOutput").ap(),
    }

    with tile.TileContext(nc) as tc:
        with (
            tc.tile_pool(name="w", bufs=1) as wpool,
            tc.tile_pool(name="res", bufs=1) as res,
            tc.tile_pool(name="io", bufs=3) as io,
            tc.tile_pool(name="pt", bufs=3) as pt_pool,
            tc.tile_pool(name="yp", bufs=2) as ypool,
            tc.tile_pool(name="psa", bufs=2, space="PSUM") as ps_a,
            tc.tile_pool(name="pspv", bufs=2, space="PSUM") as ps_pv,
        ):
            # constants / weights (loaded once)
            wq_sb = wpool.tile([P, 8, F], FP, name="wq_sb")
            wk_sb = wpool.tile([P, 8, F], FP, name="wk_sb")
            wv_sb = wpool.tile([P, 8, F], FP, name="wv_sb")
            wo_sb = wpool.tile([P, 2, E], FP, name="wo_sb")
            maskb_sb = wpool.tile([P, 16], FP, name="maskb_sb")
            bq_sb = wpool.tile([P, 2], FP, name="bq_sb")
            bk_sb = wpool.tile([P, 2], FP, name="bk_sb")
            ones64 = wpool.tile([1, D], FP, name="ones64")

            nc.sync.dma_start(wq_sb[:], dram["wq"].rearrange("(c p) f -> p c f", p=P))
            nc.sync.dma_start(wk_sb[:], dram["wk"].rearrange("(c p) f -> p c f", p=P))
            nc.sync.dma_start(wv_sb[:], dram["wv"].rearrange("(c p) f -> p c f", p=P))
            nc.sync.dma_start(wo_sb[:], dram["wo"].rearrange("(c p) f -> p c f", p=P))
            nc.sync.dma_start(maskb_sb[:], dram["maskb"][:])
            nc.sync.dma_start(bq_sb[:], dram["bq2"][:])
            nc.sync.dma_start(bk_sb[:], dram["bk2"][:])

            nc.vector.memset(ones64[:], 1.0)

            pools = {
                "consts": (wq_sb, wk_sb, wv_sb, wo_sb, maskb_sb, bq_sb, bk_sb, ones64),
                "res": res, "io": io, "pt": pt_pool, "y": ypool,
                "ps_a": ps_a, "ps_pv": ps_pv,
            }
            if loop_n is not None:
                with tc.For_i(0, loop_n, 1):
                    _build_body(nc, tc, pools, dram, 0)
            else:
                for rep in range(repeats):
                    _build_body(nc, tc, pools, dram, rep)

    nc.compile()
    return nc


def _make_in_maps(query, key, value, key_padding_mask, Wq, bq, Wk, bk, Wv, bv, Wo, bo):
    """Host-side sharding: returns (in_maps list of 8 dicts, bias_total)."""
    f32 = np.float32
    query = np.asarray(query, f32)
    key = np.asarray(key, f32)
    value = np.asarray(value, f32)
    mask = np.asarray(key_padding_mask)
    Wq, Wk, Wv, Wo = (np.asarray(w, f32) for w in (Wq, Wk, Wv, Wo))
    bq, bk, bv, bo = (np.asarray(b, f32) for b in (bq, bk, bv, bo))

    c = np.ascontiguousarray
    qT = [c(query[b].T) for b in range(B)]
    kTb = [c(key[b].T) for b in range(B)]
    vTb = [c(value[b].T) for b in range(B)]
    maskb = [
        c(np.where(mask[b], f32(MASK_BIAS), f32(0.0)).astype(f32).reshape(16, P).T)
        for b in range(B)
    ]
    in_maps = []
    for core in range(N_CORES):
        b, g = divmod(core, G)
        sl = slice(g * F, (g + 1) * F)
        in_maps.append({
            "qT": qT[b], "kT": kTb[b], "vT": vTb[b],
            "wq": c(Wq[sl, :].T), "wk": c(Wk[sl, :].T), "wv": c(Wv[sl, :].T),
            "wo": c(Wo[:, sl].T),
            "bq2": c(bq[sl].reshape(2, P).T), "bk2": c(bk[sl].reshape(2, P).T),
            "maskb": maskb[b],
        })
    bias_total = (bo + bv @ Wo.T).astype(f32)
    return in_maps, bias_total


_CACHED_NC = None


def kernel(**inputs) -> np.ndarray:
    global _CACHED_NC
    if _CACHED_NC is None:
        _CACHED_NC = build_module(repeats=1)
    nc = _CACHED_NC

    in_maps, bias_total = _make_in_maps(**inputs)
    res = bass_utils.run_bass_kernel_spmd(
        nc, in_maps, core_ids=list(range(N_CORES)), trace=False
    )
    out = np.empty((B, Q, E), np.float32)
    for b in range(B):
        acc = res.results[b * G]["y"].astype(np.float32)
        for g in range(1, G):
            acc = acc + res.results[b * G + g]["y"]
        out[b] = acc + bias_total[None, :]
    return out
